# revision 1
# baseline (speedup 1.0000x reference)
"""Trainium2 8-core GQA attention kernel (tensor-parallel over heads).

Strategy (8 NeuronCores, SPMD):
  - Core c owns q-heads [4c..4c+4) and kv-head c (GQA groups stay aligned).
  - Phases A (qkv projection + RoPE) and B (attention) are merged per token
    chunk: causality means chunk (b,qc) only attends k-chunks <= qc, so the
    attention for a chunk is emitted right after its projection and the Tile
    scheduler fills attention's exp-latency stalls with projection matmuls.
  - qkvT = wqkv_c^T @ x^T is computed feature-major so Q^T/K^T land in
    [head_dim, tokens] layout; RoPE applied with partition-shifted multiply-adds.
  - Attention scores are computed transposed (S^T[k,q]) so exp(S^T) feeds the
    PV matmul directly (lhsT = V[k,d]) with zero P transposes; fully-masked
    causal blocks are skipped; partially-masked blocks get a multiplicative
    {0,1} bf16 mask post-exp; denominators for all 4 heads accumulate into one
    [4, 512] PSUM row-set via indicator-column matmuls, 4 exp-blocks per
    matmul (summed on DVE); normalization is deferred to the output.
  - The AllGather of attention outputs is split into 8 token-chunk collectives
    issued as soon as each chunk's attention completes; phase C (the wo
    projection, out^T = wo_c^T @ attn^T) runs as a solid block at the end,
    by which time all AllGathers have long completed.
  - Host: shards/casts inputs, transposes x, concatenates output slices.
All PE math in bf16 (f32 PSUM accumulation).
"""

import numpy as np
import ml_dtypes

import concourse.bass as bass
import concourse.mybir as mybir
import concourse.tile as tile
from concourse import bacc
from concourse.bass_utils import run_bass_kernel_spmd

BF16 = mybir.dt.bfloat16
F32 = mybir.dt.float32
HD = 128            # head dim
HHD = HD // 2       # rope half
P = 128             # partitions
QCH = 512           # q-chunk / token-chunk size
KT = 128            # k tile (partition dim)
SCALE = 1.0 / np.sqrt(HD)


def build_graph(NB, S, D, HPC, NCORES, block_cls, n_mixed, qc_mask):
    """Build the per-core SPMD graph.

    block_cls[(qc, kt)] -> 'full' | 'skip' | int (mixed-mask slot index)
    qc_mask[qc] -> (first_slot, count) of that q-chunk's mixed-mask slots
    """
    TOK = NB * S
    QF = HPC * HD           # q features per core
    FLOC = QF + 2 * HD      # local qkv features (q + k + v)
    MT = FLOC // P          # feature tiles (q tiles + 1 k + 1 v)
    KD = D // P             # contraction tiles over model dim
    NQC = S // QCH          # q chunks per batch
    NKT = S // KT           # k tiles per batch
    KTC = QCH // KT         # k tiles per token chunk
    ODPC = D // NCORES      # output dims per core
    NCHK = TOK // QCH       # token chunks overall
    n_mask = max(n_mixed, 1)

    nc = bacc.Bacc("TRN2", target_bir_lowering=False, debug=False,
                   num_devices=NCORES)

    xt_d = nc.dram_tensor("xt", [D, TOK], BF16, kind="ExternalInput").ap()
    wqkv_d = nc.dram_tensor("wqkv", [D, FLOC], BF16, kind="ExternalInput").ap()
    wo_d = nc.dram_tensor("wo", [D, ODPC], BF16, kind="ExternalInput").ap()
    sc_d = nc.dram_tensor("sincos2", [P, 2 * S], BF16, kind="ExternalInput").ap()
    mask_d = nc.dram_tensor("maskblk", [n_mask * P, QCH], BF16,
                            kind="ExternalInput").ap()
    eye_d = nc.dram_tensor("eye", [HPC, HPC * P], BF16,
                           kind="ExternalInput").ap()
    out_d = nc.dram_tensor("out", [ODPC, TOK], F32, kind="ExternalOutput").ap()

    with tile.TileContext(nc) as tc:
        with tc.tile_pool(name="persist", bufs=1) as persist, \
             tc.tile_pool(name="dram", bufs=1, space="DRAM") as dram:
            qkvT = persist.tile([P, MT, TOK], BF16)
            v_kd = persist.tile([P, NB * NKT, HD], BF16)
            ident = persist.tile([P, P], BF16)
            nc.gpsimd.memset(ident[:], 0.0)
            nc.gpsimd.affine_select(
                out=ident[:], in_=ident[:],
                compare_op=mybir.AluOpType.not_equal, fill=1.0, base=0,
                pattern=[[-1, P]], channel_multiplier=1)
            # indicator columns/rows for per-head denominator batching
            ecol = persist.tile([P, HPC, HPC], BF16)   # [:, h, :] = e_h cols
            erow = persist.tile([HPC, HPC, P], BF16)   # [:, h, :] = e_h rows
            nc.vector.memset(ecol[:], 0.0)
            for h in range(HPC):
                nc.vector.memset(ecol[:, h, h:h + 1], 1.0)
            nc.scalar.dma_start(erow[:], eye_d[:])

            bounce = [dram.tile([QF, QCH], BF16, name=f"bnc{ci}")
                      for ci in range(NCHK)]
            agc = [dram.tile([QF * NCORES, QCH], BF16, name=f"agc{ci}",
                             addr_space="Shared" if NCORES > 4 else "Local")
                   for ci in range(NCHK)]

            # ---------- merged phases A (projection+RoPE) and B (attention) --
            with tc.tile_pool(name="phbw", bufs=3) as phbw, \
                 tc.tile_pool(name="phbm", bufs=2) as phbm, \
                 tc.tile_pool(name="psab", bufs=1, space="PSUM") as psab:
              with tc.tile_pool(name="pha", bufs=1) as pha, \
                 tc.tile_pool(name="phax", bufs=2) as phax, \
                 tc.tile_pool(name="phat", bufs=2) as phat:
                KH = KD // 2
                wq_sb = pha.tile([P, KD, FLOC], BF16)

                def load_xt(col0, half, tagname):
                    xt_sb = phax.tile([P, KH, QCH], BF16, tag="xt",
                                      name=tagname)
                    nc.sync.dma_start(
                        xt_sb[:],
                        xt_d[half * KH * P:(half + 1) * KH * P,
                             col0:col0 + QCH]
                        .rearrange("(ko p) t -> p ko t", p=P))
                    return xt_sb

                for ko in range(2):
                    nc.sync.dma_start(
                        wq_sb[:, ko, :], wqkv_d[ko * P:(ko + 1) * P, :])
                xt_first = phax.tile([P, KH, QCH], BF16, tag="xt",
                                     name="xtf")
                nspl = 4 if KH % 4 == 0 else 1
                qk = KH // nspl
                for q4 in range(nspl):
                    nc.sync.dma_start(
                        xt_first[:, q4 * qk:(q4 + 1) * qk, :],
                        xt_d[q4 * qk * P:(q4 + 1) * qk * P, 0:QCH]
                        .rearrange("(ko p) t -> p ko t", p=P))
                for ko in range(2, KD // 2):
                    nc.sync.dma_start(
                        wq_sb[:, ko, :], wqkv_d[ko * P:(ko + 1) * P, :])
                xt_first1 = load_xt(0, 1, "xtf1")
                for ko in range(KD // 2, KD):
                    nc.sync.dma_start(
                        wq_sb[:, ko, :], wqkv_d[ko * P:(ko + 1) * P, :])
                sc_sb = pha.tile([P, 2 * S], BF16)
                nc.scalar.dma_start(sc_sb[:], sc_d[:])
                cosT = sc_sb[:, 0:S]
                sinT = sc_sb[:, S:2 * S]

                m_groups = [list(range(g, min(g + 3, MT)))
                            for g in range(0, MT, 3)]

                def proj_chunk(b, cb):
                    ch = b * (S // QCH) + cb
                    col0 = ch * QCH
                    s0 = col0 % S
                    for gi, grp in enumerate(m_groups):
                        pss = {m: psab.tile([P, QCH], F32, tag="pa", bufs=5,
                                            name=f"pa{ch}_{m}")
                               for m in grp}
                        for half in range(2):
                            if ch == 0:
                                xt_sb = xt_first if half == 0 else xt_first1
                            else:
                                xt_sb = load_xt(col0, half,
                                                f"xt{ch}_{gi}_{half}")
                            for k in range(KH):
                                kg = half * KH + k
                                for m in grp:
                                    nc.tensor.matmul(
                                        pss[m][:],
                                        wq_sb[:, kg, m * P:(m + 1) * P],
                                        xt_sb[:, k, :],
                                        start=(kg == 0), stop=(kg == KD - 1))
                        for m in grp:
                            dst = qkvT[:, m, col0:col0 + QCH]
                            if m == MT - 1:  # v
                                nc.vector.tensor_copy(dst, pss[m][:])
                                continue
                            t1 = phat.tile([P, QCH], F32, tag="t1",
                                           name=f"t1_{ch}_{m}")
                            t2 = phat.tile([P, QCH], F32, tag="t2",
                                           name=f"t2_{ch}_{m}")
                            nc.vector.tensor_mul(t1[:], pss[m][:],
                                                 cosT[:, s0:s0 + QCH])
                            nc.vector.tensor_mul(t2[0:HHD, :],
                                                 pss[m][HHD:P, :],
                                                 sinT[0:HHD, s0:s0 + QCH])
                            nc.vector.tensor_mul(t2[HHD:P, :],
                                                 pss[m][0:HHD, :],
                                                 sinT[HHD:P, s0:s0 + QCH])
                            nc.vector.tensor_add(dst, t1[:], t2[:])
                    # V^T -> V via PE transposes for this chunk's k tiles
                    for kt in range(cb * KTC, (cb + 1) * KTC):
                        pt_ps = psab.tile([P, P], BF16, tag="pa", bufs=5,
                                          name=f"vt{b}_{kt}")
                        nc.tensor.transpose(
                            pt_ps[:],
                            qkvT[:, MT - 1,
                                 b * S + kt * KT:b * S + (kt + 1) * KT],
                            ident[:])
                        nc.vector.tensor_copy(v_kd[:, b * NKT + kt, :],
                                              pt_ps[:])

                def attn_chunk(ci, b, qc):
                    kts = [kt for kt in range(NKT)
                           if block_cls[(qc, kt)] != 'skip']
                    q0 = b * S + qc * QCH
                    mfirst, mcnt = qc_mask.get(qc, (0, 0))
                    if mcnt:
                        mk = phbm.tile([P, mcnt, QCH], BF16, tag="mk",
                                       name=f"mk{ci}")
                        nc.scalar.dma_start(
                            mk[:],
                            mask_d[mfirst * P:(mfirst + mcnt) * P, :]
                            .rearrange("(mb p) q -> p mb q", p=P))
                    d_ps = psab.tile([HPC, QCH], F32, tag="pa", bufs=5,
                                     name=f"den{ci}")
                    o_tiles = {}
                    for h in range(HPC):
                        o_ps = psab.tile([P, QCH], F32, tag="outT", bufs=1,
                                         name=f"o{ci}_{h}")
                        for i, kt in enumerate(kts):
                            st = psab.tile([P, QCH], F32, tag="st", bufs=2,
                                           name=f"st{ci}_{h}_{i}")
                            nc.tensor.matmul(
                                st[:],
                                qkvT[:, HPC,
                                     b * S + kt * KT:b * S + (kt + 1) * KT],
                                qkvT[:, h, q0:q0 + QCH],
                                start=True, stop=True)
                            pt = phbw.tile([P, QCH], BF16, tag="pt", bufs=6,
                                           name=f"pt{ci}_{h}_{i}")
                            nc.scalar.activation(
                                pt[:], st[:],
                                mybir.ActivationFunctionType.Exp,
                                bias=0.0, scale=float(SCALE))
                            cls = block_cls[(qc, kt)]
                            if cls != 'full':
                                nc.vector.tensor_mul(pt[:], pt[:],
                                                     mk[:, cls - mfirst, :])
                            first, last = (i == 0), (i == len(kts) - 1)
                            nc.tensor.matmul(
                                o_ps[:], v_kd[:, b * NKT + kt, :], pt[:],
                                start=first, stop=last)
                            # group up to 4 exp blocks per denominator matmul
                            gpos = i % 4
                            if gpos == 0:
                                dacc, dacc_n = pt, 1
                            else:
                                if dacc_n == 1:
                                    dsum = phbw.tile([P, QCH], BF16,
                                                     tag="dsum", bufs=3,
                                                     name=f"ds{ci}_{h}_{i}")
                                    nc.vector.tensor_add(dsum[:], dacc[:],
                                                         pt[:])
                                    dacc = dsum
                                else:
                                    nc.vector.tensor_add(dacc[:], dacc[:],
                                                         pt[:])
                                dacc_n += 1
                            if gpos == 3 or last:
                                nc.tensor.matmul(
                                    d_ps[:], ecol[:, h, :], dacc[:],
                                    start=(i < 4 and h == 0),
                                    stop=(last and h == HPC - 1))
                        o_sb = phbw.tile([P, QCH], BF16, tag="osbuf", bufs=4,
                                         name=f"ou{ci}_{h}")
                        nc.vector.tensor_copy(o_sb[:], o_ps[:])
                        o_tiles[h] = o_sb
                    inv = phbw.tile([HPC, QCH], F32, tag="inv",
                                    name=f"inv{ci}")
                    nc.vector.reciprocal(inv[:], d_ps[:])
                    invb = phbw.tile([HPC, QCH], BF16, tag="invb",
                                     name=f"invb{ci}")
                    nc.vector.tensor_copy(invb[:], inv[:])
                    for h in range(HPC):
                        bc_ps = psab.tile([P, QCH], F32, tag="st", bufs=2,
                                          name=f"bc{ci}_{h}")
                        nc.tensor.matmul(bc_ps[:], erow[:, h, :], invb[:],
                                         start=True, stop=True)
                        bcc = phbw.tile([P, QCH], BF16, tag="bcc", bufs=2,
                                        name=f"bcc{ci}_{h}")
                        nc.vector.tensor_copy(bcc[:], bc_ps[:])
                        at = phbw.tile([P, QCH], BF16, tag="at", bufs=2,
                                       name=f"at{ci}_{h}")
                        nc.vector.tensor_mul(at[:], o_tiles[h][:], bcc[:])
                        nc.scalar.dma_start(
                            bounce[ci][h * P:(h + 1) * P, :], at[:])

                # a q-chunk's attention can only run once every k-chunk it
                # attends is projected (for causal masks: its own chunk)
                def max_kchunk(qc):
                    kts = [kt for kt in range(NKT)
                           if block_cls[(qc, kt)] != 'skip']
                    return (max(kts) // KTC) if kts else 0

                # emission schedule; the very last attention chunk is
                # deferred past the proj-pool close so phase C's loads and
                # first matmuls can fill its exp-latency stalls
                sched = []
                for b in range(NB):
                    done = set()
                    for cb in range(S // QCH):
                        ready = [qc for qc in range(NQC)
                                 if qc not in done and max_kchunk(qc) <= cb]
                        done.update(ready)
                        sched.append((b, cb, ready))
                deferred = None
                for b, cb, ready in reversed(sched):
                    if ready:
                        deferred = (b, ready[-1])
                        ready.pop()
                        break

                def emit_attn(b, qc):
                    ci = b * NQC + qc
                    attn_chunk(ci, b, qc)
                    nc.gpsimd.collective_compute(
                        "AllGather", mybir.AluOpType.bypass,
                        replica_groups=[list(range(NCORES))],
                        ins=[bounce[ci].opt()], outs=[agc[ci].opt()])

                for b, cb, ready in sched:
                    proj_chunk(b, cb)
                    for qc in ready:
                        emit_attn(b, qc)

              # -------------- Phase C: out^T = wo_c^T @ attn^T ---------------
              with tc.tile_pool(name="phc", bufs=1) as phc, \
                 tc.tile_pool(name="phcx", bufs=2) as phcx, \
                 tc.tile_pool(name="phco", bufs=2) as phco:
                wo_sb = phc.tile([P, KD, ODPC], BF16)
                for ko in range(KD):
                    eng = nc.sync if ko % 2 == 0 else nc.scalar
                    eng.dma_start(
                        wo_sb[:, ko, :], wo_d[ko * P:(ko + 1) * P, :])
                if deferred is not None:
                    emit_attn(*deferred)
                for ci in range(NCHK):
                    tok0 = ci * QCH
                    agt = phcx.tile([P, KD, QCH], BF16, tag="agt",
                                    name=f"agt{ci}")
                    kh2 = KD // 2
                    for half in range(2):
                        nc.sync.dma_start(
                            agt[:, half * kh2:(half + 1) * kh2, :],
                            agc[ci][half * kh2 * P:(half + 1) * kh2 * P, :]
                            .rearrange("(ko p) t -> p ko t", p=P))
                    for md in range(ODPC // P):
                        po = psab.tile([P, QCH], F32, tag="pa", bufs=5,
                                       name=f"po{ci}_{md}")
                        for kf in range(KD):
                            nc.tensor.matmul(
                                po[:],
                                wo_sb[:, kf, md * P:(md + 1) * P],
                                agt[:, kf, :],
                                start=(kf == 0), stop=(kf == KD - 1))
                        osb = phco.tile([P, QCH], F32, tag="osb",
                                        name=f"osb{ci}_{md}")
                        nc.vector.tensor_copy(osb[:], po[:])
                        nc.sync.dma_start(
                            out_d[md * P:(md + 1) * P,
                                  tok0:tok0 + QCH], osb[:])

    nc.compile()
    return nc


def _host_prep(x, wqkv, wo, sincos, full_causal_mask, start_pos,
               NB, S, D, HPC, NCORES):
    """Shard, cast, and lay out inputs; classify mask blocks."""
    bf16 = ml_dtypes.bfloat16
    TOK = NB * S
    H = HPC * NCORES
    QF = HPC * HD
    NQC = S // QCH
    NKT = S // KT
    ODPC = D // NCORES
    q_sz = H * HD

    xt = np.ascontiguousarray(x.reshape(TOK, D).T).astype(bf16)

    # effective mask: [q, k] (batch-shared), incl. the cache-validity term
    m_eff = np.asarray(full_causal_mask[0, 0], dtype=bool)
    m_eff = m_eff[start_pos:start_pos + S, :S].copy()
    valid = np.arange(S) < (start_pos + S)
    m_eff &= valid[None, :]

    block_cls = {}
    mixed_blocks = []
    qc_mask = {}
    for qc in range(NQC):
        first = len(mixed_blocks)
        for kt in range(NKT):
            blk = m_eff[qc * QCH:(qc + 1) * QCH, kt * KT:(kt + 1) * KT]
            if blk.all():
                block_cls[(qc, kt)] = 'full'
            elif not blk.any():
                block_cls[(qc, kt)] = 'skip'
            else:
                block_cls[(qc, kt)] = len(mixed_blocks)
                mixed_blocks.append(
                    np.ascontiguousarray(blk.T.astype(np.float32)))  # [k, q]
        cnt = len(mixed_blocks) - first
        if cnt:
            qc_mask[qc] = (first, cnt)
    n_mixed = len(mixed_blocks)
    if n_mixed:
        maskblk = np.concatenate(mixed_blocks, axis=0).astype(bf16)
    else:
        maskblk = np.zeros((P, QCH), dtype=bf16)

    # rope tables, transposed + duplicated halves; sin rows 0:64 negated
    sc = np.asarray(sincos[start_pos:start_pos + S], dtype=np.float32)
    sin, cos = sc[:, :HHD], sc[:, HHD:]
    cosT2 = np.concatenate([cos.T, cos.T], axis=0)           # [128, S]
    sinT2 = np.concatenate([-sin.T, sin.T], axis=0)          # [128, S]
    sincos2 = np.concatenate([cosT2, sinT2], axis=1).astype(bf16)

    eye = np.zeros((HPC, HPC, P), dtype=bf16)
    for h in range(HPC):
        eye[h, h, :] = 1
    eye = eye.reshape(HPC, HPC * P)

    in_maps = []
    for c in range(NCORES):
        qcols = np.asarray(wqkv[:, c * QF:(c + 1) * QF])
        kcols = np.asarray(wqkv[:, q_sz + c * HD:q_sz + (c + 1) * HD])
        vcols = np.asarray(
            wqkv[:, q_sz + NCORES * HD + c * HD:
                 q_sz + NCORES * HD + (c + 1) * HD])
        wqkv_c = np.concatenate([qcols, kcols, vcols], axis=1).astype(bf16)
        wo_c = np.ascontiguousarray(
            np.asarray(wo[:, c * ODPC:(c + 1) * ODPC])).astype(bf16)
        in_maps.append({
            "xt": xt, "wqkv": wqkv_c, "wo": wo_c,
            "sincos2": sincos2, "maskblk": maskblk, "eye": eye,
        })
    return in_maps, block_cls, n_mixed, qc_mask


_CACHE = {}


def run_distributed(x, wqkv, wo, sincos, full_causal_mask, start_pos,
                    NB, S, D, HPC, NCORES, trace=False, tmpdir=None):
    in_maps, block_cls, n_mixed, qc_mask = _host_prep(
        x, wqkv, wo, sincos, full_causal_mask, start_pos,
        NB, S, D, HPC, NCORES)
    key = (NB, S, D, HPC, NCORES,
           tuple(sorted((k, v) for k, v in block_cls.items())))
    if key not in _CACHE:
        _CACHE[key] = build_graph(NB, S, D, HPC, NCORES, block_cls, n_mixed,
                                  qc_mask)
    nc = _CACHE[key]
    res = run_bass_kernel_spmd(nc, in_maps, list(range(NCORES)), trace=trace,
                               tmpdir=tmpdir)
    TOK = NB * S
    out = np.empty((TOK, D), dtype=np.float32)
    ODPC = D // NCORES
    for c in range(NCORES):
        out[:, c * ODPC:(c + 1) * ODPC] = res.results[c]["out"].T
    return out.reshape(NB, S, D), res


def kernel(x, wqkv, wo, sincos, cache_k, cache_v, full_causal_mask,
           start_pos) -> np.ndarray:
    x = np.asarray(x)
    start_pos = int(np.asarray(start_pos))
    B, S_, D_ = x.shape
    assert start_pos == 0, "prefill-only kernel (seq fills the whole cache)"
    out, _ = run_distributed(
        x, np.asarray(wqkv), np.asarray(wo), np.asarray(sincos),
        np.asarray(full_causal_mask), start_pos,
        NB=B, S=S_, D=D_, HPC=4, NCORES=8)
    return out



# revision 9
# speedup vs baseline: 1.0721x; 1.0721x over previous
"""Trainium2 8-core GQA attention kernel (tensor-parallel over heads).

Strategy (8 NeuronCores, SPMD):
  - Core c owns q-heads [4c..4c+4) and kv-head c (GQA groups stay aligned).
  - Phases A (qkv projection + RoPE) and B (attention) are merged per token
    chunk: causality means chunk (b,qc) only attends k-chunks <= qc, so the
    attention for a chunk is emitted right after its projection and the Tile
    scheduler fills attention's exp-latency stalls with projection matmuls.
  - qkvT = wqkv_c^T @ x^T is computed feature-major so Q^T/K^T land in
    [head_dim, tokens] layout; RoPE applied with partition-shifted multiply-adds.
  - Attention scores are computed transposed (S^T[k,q]) so exp(S^T) feeds the
    PV matmul directly (lhsT = V[k,d]) with zero P transposes; fully-masked
    causal blocks are skipped; partially-masked blocks get a multiplicative
    {0,1} bf16 mask post-exp; denominators for all 4 heads accumulate into one
    [4, 512] PSUM row-set via indicator-column matmuls, 4 exp-blocks per
    matmul (summed on DVE); normalization is deferred to the output.
  - The AllGather of attention outputs is split into 8 token-chunk collectives
    issued as soon as each chunk's attention completes; phase C (the wo
    projection, out^T = wo_c^T @ attn^T) runs as a solid block at the end,
    by which time all AllGathers have long completed.
  - Host: shards/casts inputs, transposes x, concatenates output slices.
All PE math in bf16 (f32 PSUM accumulation).
"""

import numpy as np
import ml_dtypes

import concourse.bass as bass
import concourse.mybir as mybir
import concourse.tile as tile
from concourse import bacc
from concourse.bass_utils import run_bass_kernel_spmd

BF16 = mybir.dt.bfloat16
F32 = mybir.dt.float32
HD = 128            # head dim
HHD = HD // 2       # rope half
P = 128             # partitions
QCH = 512           # q-chunk / token-chunk size
KT = 128            # k tile (partition dim)
SCALE = 1.0 / np.sqrt(HD)


def build_graph(NB, S, D, HPC, NCORES, block_cls, n_mixed, qc_mask):
    """Build the per-core SPMD graph.

    block_cls[(qc, kt)] -> 'full' | 'skip' | int (mixed-mask slot index)
    qc_mask[qc] -> (first_slot, count) of that q-chunk's mixed-mask slots
    """
    TOK = NB * S
    QF = HPC * HD           # q features per core
    FLOC = QF + 2 * HD      # local qkv features (q + k + v)
    MT = FLOC // P          # feature tiles (q tiles + 1 k + 1 v)
    KD = D // P             # contraction tiles over model dim
    NQC = S // QCH          # q chunks per batch
    NKT = S // KT           # k tiles per batch
    KTC = QCH // KT         # k tiles per token chunk
    ODPC = D // NCORES      # output dims per core
    NCHK = TOK // QCH       # token chunks overall
    n_mask = max(n_mixed, 1)

    nc = bacc.Bacc("TRN2", target_bir_lowering=False, debug=False,
                   num_devices=NCORES)

    xt_d = nc.dram_tensor("xt", [D, TOK], BF16, kind="ExternalInput").ap()
    wqkv_d = nc.dram_tensor("wqkv", [D, FLOC], BF16, kind="ExternalInput").ap()
    wo_d = nc.dram_tensor("wo", [D, ODPC], BF16, kind="ExternalInput").ap()
    sc_d = nc.dram_tensor("sincos2", [P, 2 * S], BF16, kind="ExternalInput").ap()
    mask_d = nc.dram_tensor("maskblk", [n_mask * P, QCH], BF16,
                            kind="ExternalInput").ap()
    eye_d = nc.dram_tensor("eye", [HPC, HPC * P], BF16,
                           kind="ExternalInput").ap()
    out_d = nc.dram_tensor("out", [ODPC, TOK], F32, kind="ExternalOutput").ap()

    with tile.TileContext(nc) as tc:
        with tc.tile_pool(name="persist", bufs=1) as persist, \
             tc.tile_pool(name="dram", bufs=1, space="DRAM") as dram:
            qkvT = persist.tile([P, MT, TOK], BF16)
            v_kd = persist.tile([P, NB * NKT, HD], BF16)
            ident = persist.tile([P, P], BF16)
            nc.gpsimd.memset(ident[:], 0.0)
            nc.gpsimd.affine_select(
                out=ident[:], in_=ident[:],
                compare_op=mybir.AluOpType.not_equal, fill=1.0, base=0,
                pattern=[[-1, P]], channel_multiplier=1)
            # indicator columns/rows for per-head denominator batching
            ecol = persist.tile([P, HPC, HPC], BF16)   # [:, h, :] = e_h cols
            erow = persist.tile([HPC, HPC, P], BF16)   # [:, h, :] = e_h rows
            nc.vector.memset(ecol[:], 0.0)
            for h in range(HPC):
                nc.vector.memset(ecol[:, h, h:h + 1], 1.0)
            nc.scalar.dma_start(erow[:], eye_d[:])

            bounce = [dram.tile([QF, QCH], BF16, name=f"bnc{ci}")
                      for ci in range(NCHK)]
            agc = [dram.tile([QF * NCORES, QCH], BF16, name=f"agc{ci}",
                             addr_space="Shared" if NCORES > 4 else "Local")
                   for ci in range(NCHK)]

            # ---------- merged phases A (projection+RoPE) and B (attention) --
            with tc.tile_pool(name="phbw", bufs=3) as phbw, \
                 tc.tile_pool(name="phbm", bufs=2) as phbm, \
                 tc.tile_pool(name="psab", bufs=1, space="PSUM") as psab:
              with tc.tile_pool(name="pha", bufs=1) as pha, \
                 tc.tile_pool(name="phax", bufs=2) as phax, \
                 tc.tile_pool(name="phat", bufs=2) as phat:
                KH = KD // 2
                wq_sb = pha.tile([P, KD, FLOC], BF16)

                def load_xt(col0, half, tagname):
                    xt_sb = phax.tile([P, KH, QCH], BF16, tag="xt",
                                      name=tagname)
                    nc.sync.dma_start(
                        xt_sb[:],
                        xt_d[half * KH * P:(half + 1) * KH * P,
                             col0:col0 + QCH]
                        .rearrange("(ko p) t -> p ko t", p=P))
                    return xt_sb

                # startup: interleave weight and x loads so the first proj
                # matmuls (which need wq[kg] + xt[kg] in order) start ASAP
                xt_first = phax.tile([P, KH, QCH], BF16, tag="xt",
                                     name="xtf")
                nspl = 4 if KH % 4 == 0 else 1
                qk = KH // nspl
                nc.sync.dma_start(wq_sb[:, 0, :], wqkv_d[0:P, :])
                for q4 in range(nspl):
                    nc.sync.dma_start(
                        xt_first[:, q4 * qk:(q4 + 1) * qk, :],
                        xt_d[q4 * qk * P:(q4 + 1) * qk * P, 0:QCH]
                        .rearrange("(ko p) t -> p ko t", p=P))
                    k0 = 1 + q4 * 4
                    for ko in range(k0, k0 + 4):
                        nc.scalar.dma_start(
                            wq_sb[:, ko, :], wqkv_d[ko * P:(ko + 1) * P, :])
                xt_first1 = load_xt(0, 1, "xtf1")
                for ko in range(1 + 4 * nspl, KD):
                    nc.scalar.dma_start(
                        wq_sb[:, ko, :], wqkv_d[ko * P:(ko + 1) * P, :])
                sc_sb = pha.tile([P, 2 * S], BF16)
                nc.scalar.dma_start(sc_sb[:], sc_d[:])
                cosT = sc_sb[:, 0:S]
                sinT = sc_sb[:, S:2 * S]

                m_groups = [list(range(g, min(g + 3, MT)))
                            for g in range(0, MT, 3)]

                def proj_chunk(b, cb):
                    ch = b * (S // QCH) + cb
                    col0 = ch * QCH
                    s0 = col0 % S
                    for gi, grp in enumerate(m_groups):
                        pss = {m: psab.tile([P, QCH], F32, tag="pa", bufs=4,
                                            name=f"pa{ch}_{m}")
                               for m in grp}
                        for half in range(2):
                            if ch == 0:
                                xt_sb = xt_first if half == 0 else xt_first1
                            else:
                                xt_sb = load_xt(col0, half,
                                                f"xt{ch}_{gi}_{half}")
                            for k in range(KH):
                                kg = half * KH + k
                                for m in grp:
                                    nc.tensor.matmul(
                                        pss[m][:],
                                        wq_sb[:, kg, m * P:(m + 1) * P],
                                        xt_sb[:, k, :],
                                        start=(kg == 0), stop=(kg == KD - 1))
                        for m in grp:
                            dst = qkvT[:, m, col0:col0 + QCH]
                            if m == MT - 1:  # v
                                nc.vector.tensor_copy(dst, pss[m][:])
                                continue
                            t1 = phat.tile([P, QCH], F32, tag="t1",
                                           name=f"t1_{ch}_{m}")
                            t2 = phat.tile([P, QCH], F32, tag="t2",
                                           name=f"t2_{ch}_{m}")
                            nc.vector.tensor_mul(t1[:], pss[m][:],
                                                 cosT[:, s0:s0 + QCH])
                            nc.vector.tensor_mul(t2[0:HHD, :],
                                                 pss[m][HHD:P, :],
                                                 sinT[0:HHD, s0:s0 + QCH])
                            nc.vector.tensor_mul(t2[HHD:P, :],
                                                 pss[m][0:HHD, :],
                                                 sinT[HHD:P, s0:s0 + QCH])
                            nc.vector.tensor_add(dst, t1[:], t2[:])
                    # V^T -> V via PE transposes for this chunk's k tiles
                    for kt in range(cb * KTC, (cb + 1) * KTC):
                        pt_ps = psab.tile([P, P], BF16, tag="pa", bufs=4,
                                          name=f"vt{b}_{kt}")
                        nc.tensor.transpose(
                            pt_ps[:],
                            qkvT[:, MT - 1,
                                 b * S + kt * KT:b * S + (kt + 1) * KT],
                            ident[:])
                        nc.vector.tensor_copy(v_kd[:, b * NKT + kt, :],
                                              pt_ps[:])

                def attn_chunk(ci, b, qc):
                    kts = [kt for kt in range(NKT)
                           if block_cls[(qc, kt)] != 'skip']
                    q0 = b * S + qc * QCH
                    mfirst, mcnt = qc_mask.get(qc, (0, 0))
                    if mcnt:
                        mk = phbm.tile([P, mcnt, QCH], BF16, tag="mk",
                                       name=f"mk{ci}")
                        nc.scalar.dma_start(
                            mk[:],
                            mask_d[mfirst * P:(mfirst + mcnt) * P, :]
                            .rearrange("(mb p) q -> p mb q", p=P))
                    d_ps = psab.tile([HPC, QCH], F32, tag="den", bufs=1,
                                     name=f"den{ci}")
                    o_tiles = {}
                    for h in range(HPC):
                        o_ps = psab.tile([P, QCH], F32, tag="outT", bufs=1,
                                         name=f"o{ci}_{h}")
                        dacc = None
                        for i, kt in enumerate(kts):
                            st = psab.tile([P, QCH], F32, tag="st", bufs=2,
                                           name=f"st{ci}_{h}_{i}")
                            nc.tensor.matmul(
                                st[:],
                                qkvT[:, HPC,
                                     b * S + kt * KT:b * S + (kt + 1) * KT],
                                qkvT[:, h, q0:q0 + QCH],
                                start=True, stop=True)
                            pt = phbw.tile([P, QCH], BF16, tag="pt", bufs=6,
                                           name=f"pt{ci}_{h}_{i}")
                            nc.scalar.activation(
                                pt[:], st[:],
                                mybir.ActivationFunctionType.Exp,
                                bias=0.0, scale=float(SCALE))
                            cls = block_cls[(qc, kt)]
                            if cls != 'full':
                                nc.vector.tensor_mul(pt[:], pt[:],
                                                     mk[:, cls - mfirst, :])
                            first, last = (i == 0), (i == len(kts) - 1)
                            nc.tensor.matmul(
                                o_ps[:], v_kd[:, b * NKT + kt, :], pt[:],
                                start=first, stop=last)
                            # accumulate all exp blocks of this head on DVE;
                            # one denominator matmul per (chunk, head)
                            if i == 0:
                                dacc = pt
                            elif i == 1:
                                dsum = phbw.tile([P, QCH], BF16,
                                                 tag="dsum", bufs=2,
                                                 name=f"ds{ci}_{h}")
                                nc.vector.tensor_add(dsum[:], dacc[:], pt[:])
                                dacc = dsum
                            else:
                                nc.vector.tensor_add(dacc[:], dacc[:], pt[:])
                        nc.tensor.matmul(
                            d_ps[:], ecol[:, h, :], dacc[:],
                            start=(h == 0), stop=(h == HPC - 1))
                        o_sb = phbw.tile([P, QCH], BF16, tag="osbuf", bufs=8,
                                         name=f"ou{ci}_{h}")
                        nc.vector.tensor_copy(o_sb[:], o_ps[:])
                        o_tiles[h] = o_sb

                    def tail():
                        inv = phbw.tile([HPC, QCH], F32, tag="inv",
                                        name=f"inv{ci}")
                        nc.vector.reciprocal_approx_fast(inv[:], d_ps[:])
                        invb = phbw.tile([HPC, QCH], BF16, tag="invb",
                                         name=f"invb{ci}")
                        nc.vector.tensor_copy(invb[:], inv[:])
                        for h in range(HPC):
                            bc_ps = psab.tile([P, QCH], F32, tag="st",
                                              bufs=2, name=f"bc{ci}_{h}")
                            nc.tensor.matmul(bc_ps[:], erow[:, h, :],
                                             invb[:], start=True, stop=True)
                            bcc = phbw.tile([P, QCH], BF16, tag="bcc",
                                            bufs=2, name=f"bcc{ci}_{h}")
                            nc.vector.tensor_copy(bcc[:], bc_ps[:])
                            at = phbw.tile([P, QCH], BF16, tag="at", bufs=2,
                                           name=f"at{ci}_{h}")
                            nc.vector.tensor_mul(at[:], o_tiles[h][:],
                                                 bcc[:])
                            nc.scalar.dma_start(
                                bounce[ci][h * P:(h + 1) * P, :], at[:])
                        nc.gpsimd.collective_compute(
                            "AllGather", mybir.AluOpType.bypass,
                            replica_groups=[list(range(NCORES))],
                            ins=[bounce[ci].opt()], outs=[agc[ci].opt()])
                    return tail

                # a q-chunk's attention can only run once every k-chunk it
                # attends is projected (for causal masks: its own chunk)
                def max_kchunk(qc):
                    kts = [kt for kt in range(NKT)
                           if block_cls[(qc, kt)] != 'skip']
                    return (max(kts) // KTC) if kts else 0

                # emission schedule; the very last attention chunk is
                # deferred past the proj-pool close so phase C's loads and
                # first matmuls can fill its exp-latency stalls
                sched = []
                for b in range(NB):
                    done = set()
                    for cb in range(S // QCH):
                        ready = [qc for qc in range(NQC)
                                 if qc not in done and max_kchunk(qc) <= cb]
                        done.update(ready)
                        sched.append((b, cb, ready))
                deferred = None
                for b, cb, ready in reversed(sched):
                    if ready:
                        deferred = (b, ready[-1])
                        ready.pop()
                        break

                def emit_attn(b, qc):
                    ci = b * NQC + qc
                    return attn_chunk(ci, b, qc)

                # tails (reciprocal -> broadcast -> normalize -> AllGather)
                # are deferred past the NEXT chunk's projection matmuls so
                # the PE queue has independent work during the DVE chain
                pending = []
                for b, cb, ready in sched:
                    proj_chunk(b, cb)
                    for t in pending:
                        t()
                    pending = []
                    for qc in ready:
                        pending.append(emit_attn(b, qc))

              # -------------- Phase C: out^T = wo_c^T @ attn^T ---------------
              with tc.tile_pool(name="phc", bufs=1) as phc, \
                 tc.tile_pool(name="phcx", bufs=2) as phcx, \
                 tc.tile_pool(name="phco", bufs=2) as phco:
                wo_sb = phc.tile([P, KD, ODPC], BF16)
                for ko in range(KD):
                    eng = nc.sync if ko % 2 == 0 else nc.scalar
                    eng.dma_start(
                        wo_sb[:, ko, :], wo_d[ko * P:(ko + 1) * P, :])
                if deferred is not None:
                    pending.append(emit_attn(*deferred))
                for ci in range(NCHK):
                    tok0 = ci * QCH
                    agt = phcx.tile([P, KD, QCH], BF16, tag="agt",
                                    name=f"agt{ci}")
                    kh2 = KD // 2
                    for half in range(2):
                        nc.sync.dma_start(
                            agt[:, half * kh2:(half + 1) * kh2, :],
                            agc[ci][half * kh2 * P:(half + 1) * kh2 * P, :]
                            .rearrange("(ko p) t -> p ko t", p=P))
                    for md in range(ODPC // P):
                        po = psab.tile([P, QCH], F32, tag="pa", bufs=4,
                                       name=f"po{ci}_{md}")
                        for kf in range(KD):
                            nc.tensor.matmul(
                                po[:],
                                wo_sb[:, kf, md * P:(md + 1) * P],
                                agt[:, kf, :],
                                start=(kf == 0), stop=(kf == KD - 1))
                        osb = phco.tile([P, QCH], F32, tag="osb",
                                        name=f"osb{ci}_{md}")
                        nc.vector.tensor_copy(osb[:], po[:])
                        nc.sync.dma_start(
                            out_d[md * P:(md + 1) * P,
                                  tok0:tok0 + QCH], osb[:])
                    if pending:
                        pending.pop()()

    nc.compile()
    return nc


def _host_prep(x, wqkv, wo, sincos, full_causal_mask, start_pos,
               NB, S, D, HPC, NCORES):
    """Shard, cast, and lay out inputs; classify mask blocks."""
    bf16 = ml_dtypes.bfloat16
    TOK = NB * S
    H = HPC * NCORES
    QF = HPC * HD
    NQC = S // QCH
    NKT = S // KT
    ODPC = D // NCORES
    q_sz = H * HD

    xt = np.ascontiguousarray(x.reshape(TOK, D).T).astype(bf16)

    # effective mask: [q, k] (batch-shared), incl. the cache-validity term
    m_eff = np.asarray(full_causal_mask[0, 0], dtype=bool)
    m_eff = m_eff[start_pos:start_pos + S, :S].copy()
    valid = np.arange(S) < (start_pos + S)
    m_eff &= valid[None, :]

    block_cls = {}
    mixed_blocks = []
    qc_mask = {}
    for qc in range(NQC):
        first = len(mixed_blocks)
        for kt in range(NKT):
            blk = m_eff[qc * QCH:(qc + 1) * QCH, kt * KT:(kt + 1) * KT]
            if blk.all():
                block_cls[(qc, kt)] = 'full'
            elif not blk.any():
                block_cls[(qc, kt)] = 'skip'
            else:
                block_cls[(qc, kt)] = len(mixed_blocks)
                mixed_blocks.append(
                    np.ascontiguousarray(blk.T.astype(np.float32)))  # [k, q]
        cnt = len(mixed_blocks) - first
        if cnt:
            qc_mask[qc] = (first, cnt)
    n_mixed = len(mixed_blocks)
    if n_mixed:
        maskblk = np.concatenate(mixed_blocks, axis=0).astype(bf16)
    else:
        maskblk = np.zeros((P, QCH), dtype=bf16)

    # rope tables, transposed + duplicated halves; sin rows 0:64 negated
    sc = np.asarray(sincos[start_pos:start_pos + S], dtype=np.float32)
    sin, cos = sc[:, :HHD], sc[:, HHD:]
    cosT2 = np.concatenate([cos.T, cos.T], axis=0)           # [128, S]
    sinT2 = np.concatenate([-sin.T, sin.T], axis=0)          # [128, S]
    sincos2 = np.concatenate([cosT2, sinT2], axis=1).astype(bf16)

    eye = np.zeros((HPC, HPC, P), dtype=bf16)
    for h in range(HPC):
        eye[h, h, :] = 1
    eye = eye.reshape(HPC, HPC * P)

    in_maps = []
    for c in range(NCORES):
        qcols = np.asarray(wqkv[:, c * QF:(c + 1) * QF])
        kcols = np.asarray(wqkv[:, q_sz + c * HD:q_sz + (c + 1) * HD])
        vcols = np.asarray(
            wqkv[:, q_sz + NCORES * HD + c * HD:
                 q_sz + NCORES * HD + (c + 1) * HD])
        wqkv_c = np.concatenate([qcols, kcols, vcols], axis=1).astype(bf16)
        wo_c = np.ascontiguousarray(
            np.asarray(wo[:, c * ODPC:(c + 1) * ODPC])).astype(bf16)
        in_maps.append({
            "xt": xt, "wqkv": wqkv_c, "wo": wo_c,
            "sincos2": sincos2, "maskblk": maskblk, "eye": eye,
        })
    return in_maps, block_cls, n_mixed, qc_mask


_CACHE = {}


def run_distributed(x, wqkv, wo, sincos, full_causal_mask, start_pos,
                    NB, S, D, HPC, NCORES, trace=False, tmpdir=None):
    in_maps, block_cls, n_mixed, qc_mask = _host_prep(
        x, wqkv, wo, sincos, full_causal_mask, start_pos,
        NB, S, D, HPC, NCORES)
    key = (NB, S, D, HPC, NCORES,
           tuple(sorted((k, v) for k, v in block_cls.items())))
    if key not in _CACHE:
        _CACHE[key] = build_graph(NB, S, D, HPC, NCORES, block_cls, n_mixed,
                                  qc_mask)
    nc = _CACHE[key]
    res = run_bass_kernel_spmd(nc, in_maps, list(range(NCORES)), trace=trace,
                               tmpdir=tmpdir)
    TOK = NB * S
    out = np.empty((TOK, D), dtype=np.float32)
    ODPC = D // NCORES
    for c in range(NCORES):
        out[:, c * ODPC:(c + 1) * ODPC] = res.results[c]["out"].T
    return out.reshape(NB, S, D), res


def kernel(x, wqkv, wo, sincos, cache_k, cache_v, full_causal_mask,
           start_pos) -> np.ndarray:
    x = np.asarray(x)
    start_pos = int(np.asarray(start_pos))
    B, S_, D_ = x.shape
    assert start_pos == 0, "prefill-only kernel (seq fills the whole cache)"
    out, _ = run_distributed(
        x, np.asarray(wqkv), np.asarray(wo), np.asarray(sincos),
        np.asarray(full_causal_mask), start_pos,
        NB=B, S=S_, D=D_, HPC=4, NCORES=8)
    return out



# revision 17
# speedup vs baseline: 1.1277x; 1.0519x over previous
"""Trainium2 8-core GQA attention kernel (tensor-parallel over heads).

Strategy (8 NeuronCores, SPMD):
  - Core c owns q-heads [4c..4c+4) and kv-head c (GQA groups stay aligned).
  - Software-pipelined emission: attention for chunk c_i is emitted finely
    INTERLEAVED with the projection matmuls of chunk c_{i+1} (and the last
    chunk with phase C's matmuls), so the in-order PE queue always has
    independent work during attention's exp-latency bubbles.
  - qkvT = wqkv_c^T @ x^T computed feature-major; Q^T lives in a transient
    2-deep ring (only its own chunk's attention needs it), K^T is persistent
    [hd, TOK]; V^T is transposed per-chunk into persistent V[k,d] tiles.
  - Scores computed transposed (S^T[k,q]) so exp(S^T) feeds the PV matmul
    (lhsT = V[k,d]) with zero P transposes; fully-masked causal blocks are
    skipped; diagonal blocks are N-restricted to the valid q-range and get a
    compact [k,128] multiplicative mask on the triangle subblock only.
  - Denominators: exp blocks accumulated per head on DVE, one indicator-
    column matmul per (chunk, head) into a [4, 512] PSUM row-set; the
    normalization tail (fast approx reciprocal, gpsimd partition-broadcast
    instead of broadcast matmuls, normalize, bounce DMA + AllGather) is
    deferred into the next chunk's interleave slot.
  - Phase C (out^T = wo_c^T @ attn^T) consumes per-chunk AllGathers; wo is
    preloaded during phase A on the idle gpsimd queue.
All PE math in bf16 (f32 PSUM accumulation).
"""

import numpy as np
import ml_dtypes

import concourse.bass as bass
import concourse.mybir as mybir
import concourse.tile as tile
from concourse import bacc
from concourse.bass_utils import run_bass_kernel_spmd

BF16 = mybir.dt.bfloat16
F32 = mybir.dt.float32
HD = 128            # head dim
HHD = HD // 2       # rope half
P = 128             # partitions
QCH = 512           # q-chunk / token-chunk size
KT = 128            # k tile (partition dim)
SCALE = 1.0 / np.sqrt(HD)
RESTRICT = True     # N-restrict diagonal score/PV matmuls to valid q range


def build_graph(NB, S, D, HPC, NCORES, block_cls, n_mixed, qc_mask):
    """Build the per-core SPMD graph.

    block_cls[(qc, kt)] -> 'full' | 'skip' | int (mixed-mask slot index)
    qc_mask[qc] -> (first_slot, count) of that q-chunk's mixed-mask slots
    """
    TOK = NB * S
    QF = HPC * HD           # q features per core
    FLOC = QF + 2 * HD      # local qkv features (q + k + v)
    MT = FLOC // P          # feature tiles (q tiles + 1 k + 1 v)
    KD = D // P             # contraction tiles over model dim
    KH = KD // 2
    NQC = S // QCH          # q chunks per batch
    NKT = S // KT           # k tiles per batch
    KTC = QCH // KT         # k tiles per token chunk
    ODPC = D // NCORES      # output dims per core
    NCHK = TOK // QCH       # token chunks overall
    n_mask = max(n_mixed, 1)

    nc = bacc.Bacc("TRN2", target_bir_lowering=False, debug=False,
                   num_devices=NCORES)

    xt_d = nc.dram_tensor("xt", [D, TOK], BF16, kind="ExternalInput").ap()
    wqkv_d = nc.dram_tensor("wqkv", [D, FLOC], BF16, kind="ExternalInput").ap()
    wo_d = nc.dram_tensor("wo", [D, ODPC], BF16, kind="ExternalInput").ap()
    sc_d = nc.dram_tensor("sincos2", [P, 2 * S], BF16, kind="ExternalInput").ap()
    mask_d = nc.dram_tensor("maskblk", [n_mask * P, KT], BF16,
                            kind="ExternalInput").ap()
    eye_d = nc.dram_tensor("eye", [HPC, HPC * P], BF16,
                           kind="ExternalInput").ap()
    out_d = nc.dram_tensor("out", [ODPC, TOK], F32, kind="ExternalOutput").ap()

    with tile.TileContext(nc) as tc:
        with tc.tile_pool(name="persist", bufs=1) as persist, \
             tc.tile_pool(name="dram", bufs=1, space="DRAM") as dram:
            Kt = persist.tile([P, TOK], BF16)          # rope'd K^T, all tokens
            v_kd = persist.tile([P, NB * NKT, HD], BF16)
            wo_sb = persist.tile([P, KD, ODPC], BF16)  # preloaded during A
            mkall = persist.tile([P, n_mask, KT], BF16)
            nc.gpsimd.dma_start(
                mkall[:],
                mask_d[:].rearrange("(mb p) q -> p mb q", p=P))
            ident = persist.tile([P, P], BF16)
            nc.gpsimd.memset(ident[:], 0.0)
            nc.gpsimd.affine_select(
                out=ident[:], in_=ident[:],
                compare_op=mybir.AluOpType.not_equal, fill=1.0, base=0,
                pattern=[[-1, P]], channel_multiplier=1)
            # indicator columns/rows for per-head denominator + broadcast
            ecol = persist.tile([P, HPC, HPC], BF16)   # [:, h, :] = e_h cols
            erow = persist.tile([HPC, HPC, P], BF16)   # [:, h, :] = e_h rows
            nc.vector.memset(ecol[:], 0.0)
            for h in range(HPC):
                nc.vector.memset(ecol[:, h, h:h + 1], 1.0)
            nc.scalar.dma_start(erow[:], eye_d[:])

            bounce = [dram.tile([QF, QCH], BF16, name=f"bnc{ci}")
                      for ci in range(NCHK)]
            agc = [dram.tile([QF * NCORES, QCH], BF16, name=f"agc{ci}",
                             addr_space="Shared" if NCORES > 4 else "Local")
                   for ci in range(NCHK)]

            with tc.tile_pool(name="phbw", bufs=3) as phbw, \
                 tc.tile_pool(name="phq", bufs=2) as phq, \
                 tc.tile_pool(name="psab", bufs=1, space="PSUM") as psab:
              tails = {}

              # ---------------- attention (generator) -----------------------
              def attn_gen(ci, b, qc, qt):
                  kts = [kt for kt in range(NKT)
                         if block_cls[(qc, kt)] != 'skip']
                  d_ps = psab.tile([HPC, QCH], F32, tag="den", bufs=1,
                                   name=f"den{ci}")
                  o_tiles = {}
                  for h in range(HPC):
                      o_ps = psab.tile([P, QCH], F32, tag="outT", bufs=1,
                                       name=f"o{ci}_{h}")
                      dacc = None
                      for i, kt in enumerate(kts):
                          cls = block_cls[(qc, kt)]
                          qoff = 0
                          if RESTRICT and cls != 'full':
                              qoff = kt * KT - qc * QCH
                          st = psab.tile([P, QCH], F32, tag="st", bufs=2,
                                         name=f"st{ci}_{h}_{i}")
                          nc.tensor.matmul(
                              st[:, qoff:],
                              Kt[:, b * S + kt * KT:b * S + (kt + 1) * KT],
                              qt[:, h, qoff:],
                              start=True, stop=True)
                          yield
                          pt = phbw.tile([P, QCH], BF16, tag="pt", bufs=6,
                                         name=f"pt{ci}_{h}_{i}")
                          nc.scalar.activation(
                              pt[:, qoff:], st[:, qoff:],
                              mybir.ActivationFunctionType.Exp,
                              bias=0.0, scale=float(SCALE))
                          if cls != 'full':
                              nc.vector.tensor_mul(
                                  pt[:, qoff:qoff + KT],
                                  pt[:, qoff:qoff + KT], mkall[:, cls, :])
                          first, last = (i == 0), (i == len(kts) - 1)
                          nc.tensor.matmul(
                              o_ps[:, qoff:], v_kd[:, b * NKT + kt, :],
                              pt[:, qoff:], start=first, stop=last)
                          yield
                          if i == 0:
                              dacc = pt
                          elif i == 1:
                              dsum = phbw.tile([P, QCH], BF16, tag="dsum",
                                               bufs=2, name=f"ds{ci}_{h}")
                              nc.vector.tensor_copy(dsum[:], dacc[:])
                              nc.vector.tensor_add(
                                  dsum[:, qoff:], dsum[:, qoff:],
                                  pt[:, qoff:])
                              dacc = dsum
                          else:
                              nc.vector.tensor_add(
                                  dacc[:, qoff:], dacc[:, qoff:],
                                  pt[:, qoff:])
                      yield
                      nc.tensor.matmul(
                          d_ps[:], ecol[:, h, :], dacc[:],
                          start=(h == 0), stop=(h == HPC - 1))
                      o_sb = phbw.tile([P, QCH], BF16, tag="osbuf", bufs=8,
                                       name=f"ou{ci}_{h}")
                      nc.vector.tensor_copy(o_sb[:], o_ps[:])
                      o_tiles[h] = o_sb

                  def tail():
                      inv = phbw.tile([HPC, QCH], F32, tag="inv",
                                      name=f"inv{ci}")
                      nc.vector.reciprocal_approx_fast(inv[:], d_ps[:])
                      invb = phbw.tile([HPC, QCH], BF16, tag="invb",
                                       name=f"invb{ci}")
                      nc.vector.tensor_copy(invb[:], inv[:])
                      for h in range(HPC):
                          bc_ps = psab.tile([P, QCH], F32, tag="st",
                                            bufs=2, name=f"bc{ci}_{h}")
                          nc.tensor.matmul(bc_ps[:], erow[:, h, :],
                                           invb[:], start=True, stop=True)
                          ib = phbw.tile([P, QCH], BF16, tag="ibc", bufs=2,
                                         name=f"ib{ci}_{h}")
                          nc.vector.tensor_copy(ib[:], bc_ps[:])
                          at = phbw.tile([P, QCH], BF16, tag="at", bufs=2,
                                         name=f"at{ci}_{h}")
                          nc.vector.tensor_mul(at[:], o_tiles[h][:], ib[:])
                          nc.gpsimd.dma_start(
                              bounce[ci][h * P:(h + 1) * P, :], at[:])
                      nc.gpsimd.collective_compute(
                          "AllGather", mybir.AluOpType.bypass,
                          replica_groups=[list(range(NCORES))],
                          ins=[bounce[ci].opt()], outs=[agc[ci].opt()])
                  tails[ci] = tail

              # ---------------- projection + RoPE (generator) ---------------
              with tc.tile_pool(name="pha", bufs=1) as pha, \
                   tc.tile_pool(name="phax", bufs=2) as phax, \
                   tc.tile_pool(name="phat", bufs=2) as phat:
                wq_sb = pha.tile([P, KD, FLOC], BF16)
                sc_sb = pha.tile([P, 2 * S], BF16)
                cosT = sc_sb[:, 0:S]
                sinT = sc_sb[:, S:2 * S]

                def load_xt_half(ch, half, nspl=2):
                    col0 = ch * QCH
                    xt_sb = phax.tile([P, KH, QCH], BF16, tag="xt",
                                      name=f"xt{ch}_{half}")
                    qk = KH // nspl
                    for q4 in range(nspl):
                        r0 = (half * KH + q4 * qk) * P
                        nc.sync.dma_start(
                            xt_sb[:, q4 * qk:(q4 + 1) * qk, :],
                            xt_d[r0:r0 + qk * P, col0:col0 + QCH]
                            .rearrange("(ko p) t -> p ko t", p=P))
                    return xt_sb

                # startup: interleave wq and first-chunk x loads so the
                # first proj matmuls (needing wq[kg] + xt[kg]) start ASAP
                nc.sync.dma_start(wq_sb[:, 0, :], wqkv_d[0:P, :])
                xt0_h0 = phax.tile([P, KH, QCH], BF16, tag="xt", name="xt0_0")
                qk = KH // 4
                for q4 in range(4):
                    nc.sync.dma_start(
                        xt0_h0[:, q4 * qk:(q4 + 1) * qk, :],
                        xt_d[q4 * qk * P:(q4 + 1) * qk * P, 0:QCH]
                        .rearrange("(ko p) t -> p ko t", p=P))
                    for ko in range(1 + q4 * 4, 5 + q4 * 4):
                        nc.scalar.dma_start(
                            wq_sb[:, ko, :], wqkv_d[ko * P:(ko + 1) * P, :])
                xt0_h1 = load_xt_half(0, 1)
                for ko in range(17, KD):
                    nc.scalar.dma_start(
                        wq_sb[:, ko, :], wqkv_d[ko * P:(ko + 1) * P, :])
                nc.scalar.dma_start(sc_sb[:], sc_d[:])

                m_groups = [[0, 1], [2, 3], [4, 5]]

                def proj_gen(ci, b, cb, first=False):
                    col0 = ci * QCH
                    s0 = col0 % S
                    if first:
                        halves = [xt0_h0, xt0_h1]
                    else:
                        halves = [load_xt_half(ci, 0), load_xt_half(ci, 1)]
                    qt = phq.tile([P, HPC, QCH], BF16, tag="qt",
                                  name=f"qt{ci}")
                    vt = phq.tile([P, QCH], BF16, tag="vt", name=f"vt{ci}")
                    for grp in m_groups:
                        pss = {m: psab.tile([P, QCH], F32, tag="pa", bufs=3,
                                            name=f"pa{ci}_{m}")
                               for m in grp}
                        for half in range(2):
                            xt_sb = halves[half]
                            for k in range(KH):
                                kg = half * KH + k
                                for m in grp:
                                    nc.tensor.matmul(
                                        pss[m][:],
                                        wq_sb[:, kg, m * P:(m + 1) * P],
                                        xt_sb[:, k, :],
                                        start=(kg == 0), stop=(kg == KD - 1))
                                    yield
                        for m in grp:
                            if m == MT - 1:        # V: no rope
                                nc.vector.tensor_copy(vt[:], pss[m][:])
                                continue
                            dst = (qt[:, m, :] if m < HPC
                                   else Kt[:, col0:col0 + QCH])
                            t1 = phat.tile([P, QCH], F32, tag="t1",
                                           name=f"t1_{ci}_{m}")
                            t2 = phat.tile([P, QCH], F32, tag="t2",
                                           name=f"t2_{ci}_{m}")
                            nc.vector.tensor_mul(t1[:], pss[m][:],
                                                 cosT[:, s0:s0 + QCH])
                            nc.vector.tensor_mul(t2[0:HHD, :],
                                                 pss[m][HHD:P, :],
                                                 sinT[0:HHD, s0:s0 + QCH])
                            nc.vector.tensor_mul(t2[HHD:P, :],
                                                 pss[m][0:HHD, :],
                                                 sinT[HHD:P, s0:s0 + QCH])
                            nc.vector.tensor_add(dst, t1[:], t2[:])
                    # V^T -> V via PE transposes for this chunk's k tiles
                    for j in range(KTC):
                        pt_ps = psab.tile([P, P], BF16, tag="pa", bufs=3,
                                          name=f"vt{ci}_{j}")
                        nc.tensor.transpose(
                            pt_ps[:], vt[:, j * KT:(j + 1) * KT], ident[:])
                        yield
                        nc.vector.tensor_copy(
                            v_kd[:, b * NKT + cb * KTC + j, :], pt_ps[:])
                    qts[ci] = qt

                # ---------------- interleaved emission ------------------
                def interleave(main, filler, f):
                    """Pull f filler items per main item; return filler."""
                    acc = 0.0
                    for _ in main:
                        if filler is None:
                            continue
                        acc += f
                        while acc >= 1.0:
                            acc -= 1.0
                            if next(filler, _SENT) is _SENT:
                                filler = None
                                break
                    return filler

                _SENT = object()
                qts = {}

                def drain(gen):
                    for _ in gen:
                        pass

                chunks = [(b * NQC + qc, b, qc)
                          for b in range(NB) for qc in range(NQC)]

                def n_attn_items(qc):
                    kts = [kt for kt in range(NKT)
                           if block_cls[(qc, kt)] != 'skip']
                    return HPC * (2 * len(kts) + 1)

                # slot 0: proj(c0) solid
                drain(proj_gen(0, chunks[0][1], chunks[0][2], first=True))
                # wo preload on the idle gpsimd queue
                for ko in range(KD):
                    nc.gpsimd.dma_start(
                        wo_sb[:, ko, :], wo_d[ko * P:(ko + 1) * P, :])
                # slots 1..NCHK-1: attn(c_{i-1}) x proj(c_i)
                for i in range(1, NCHK):
                    ci, b, qc = chunks[i]
                    pci, pb, pqc = chunks[i - 1]
                    if i >= 2:
                        tails.pop(chunks[i - 2][0])()
                    pg = proj_gen(ci, b, qc)
                    ag = attn_gen(pci, pb, pqc, qts[pci])
                    f = 200.0 / n_attn_items(pqc)
                    rest = interleave(ag, pg, f)
                    if rest is not None:
                        drain(rest)

              # ------------- phase C: out^T = wo_c^T @ attn^T -------------
              # interleaved with the last chunk's attention
              with tc.tile_pool(name="phcx", bufs=3) as phcx, \
                   tc.tile_pool(name="phco", bufs=2) as phco:
                def c_gen():
                    kh2 = KD // 2
                    for ci in range(NCHK):
                        tok0 = ci * QCH
                        ahs = []
                        for half in range(2):
                            agt = phcx.tile([P, kh2, QCH], BF16, tag="agt",
                                            name=f"agt{ci}_{half}")
                            nc.sync.dma_start(
                                agt[:],
                                agc[ci][half * kh2 * P:
                                        (half + 1) * kh2 * P, :]
                                .rearrange("(ko p) t -> p ko t", p=P))
                            ahs.append(agt)
                        for md in range(ODPC // P):
                            po = psab.tile([P, QCH], F32, tag="pa", bufs=3,
                                           name=f"po{ci}_{md}")
                            for kf in range(KD):
                                nc.tensor.matmul(
                                    po[:],
                                    wo_sb[:, kf, md * P:(md + 1) * P],
                                    ahs[kf // kh2][:, kf % kh2, :],
                                    start=(kf == 0), stop=(kf == KD - 1))
                                yield
                            osb = phco.tile([P, QCH], F32, tag="osb",
                                            name=f"osb{ci}_{md}")
                            nc.vector.tensor_copy(osb[:], po[:])
                            nc.sync.dma_start(
                                out_d[md * P:(md + 1) * P,
                                      tok0:tok0 + QCH], osb[:])

                lci, lb, lqc = chunks[-1]
                tails.pop(chunks[-2][0])()
                cg = c_gen()
                ag = attn_gen(lci, lb, lqc, qts[lci])
                # cap fillers so the final C chunk (which needs the last
                # AllGather) is emitted after the last tail
                cap = (NCHK - 1) * (ODPC // P) * KD // max(
                    n_attn_items(lqc), 1)
                rest = interleave(ag, cg, min(4.0, float(cap)))
                tails.pop(lci)()
                if rest is not None:
                    drain(rest)

    nc.compile()
    return nc


def _host_prep(x, wqkv, wo, sincos, full_causal_mask, start_pos,
               NB, S, D, HPC, NCORES):
    """Shard, cast, and lay out inputs; classify mask blocks."""
    bf16 = ml_dtypes.bfloat16
    TOK = NB * S
    H = HPC * NCORES
    QF = HPC * HD
    NQC = S // QCH
    NKT = S // KT
    ODPC = D // NCORES
    q_sz = H * HD

    xt = np.ascontiguousarray(x.reshape(TOK, D).T).astype(bf16)

    # effective mask: [q, k] (batch-shared), incl. the cache-validity term
    m_eff = np.asarray(full_causal_mask[0, 0], dtype=bool)
    m_eff = m_eff[start_pos:start_pos + S, :S].copy()
    valid = np.arange(S) < (start_pos + S)
    m_eff &= valid[None, :]

    block_cls = {}
    mixed_blocks = []
    qc_mask = {}
    for qc in range(NQC):
        first = len(mixed_blocks)
        for kt in range(NKT):
            blk = m_eff[qc * QCH:(qc + 1) * QCH, kt * KT:(kt + 1) * KT]
            if blk.all():
                block_cls[(qc, kt)] = 'full'
            elif not blk.any():
                block_cls[(qc, kt)] = 'skip'
            else:
                # mixed block must be diagonal: all-zero below the valid
                # q-range, all-one above the triangle subblock
                qoff = kt * KT - qc * QCH
                assert 0 <= qoff < QCH, (qc, kt)
                blkT = blk.T  # [k, q]
                assert not blkT[:, :qoff].any()
                assert blkT[:, qoff + KT:].all()
                block_cls[(qc, kt)] = len(mixed_blocks)
                mixed_blocks.append(np.ascontiguousarray(
                    blkT[:, qoff:qoff + KT].astype(np.float32)))
        cnt = len(mixed_blocks) - first
        if cnt:
            qc_mask[qc] = (first, cnt)
    n_mixed = len(mixed_blocks)
    if n_mixed:
        maskblk = np.concatenate(mixed_blocks, axis=0).astype(bf16)
    else:
        maskblk = np.zeros((P, KT), dtype=bf16)

    eye = np.zeros((HPC, HPC, P), dtype=bf16)
    for h in range(HPC):
        eye[h, h, :] = 1
    eye = eye.reshape(HPC, HPC * P)

    # rope tables, transposed + duplicated halves; sin rows 0:64 negated
    sc = np.asarray(sincos[start_pos:start_pos + S], dtype=np.float32)
    sin, cos = sc[:, :HHD], sc[:, HHD:]
    cosT2 = np.concatenate([cos.T, cos.T], axis=0)           # [128, S]
    sinT2 = np.concatenate([-sin.T, sin.T], axis=0)          # [128, S]
    sincos2 = np.concatenate([cosT2, sinT2], axis=1).astype(bf16)

    in_maps = []
    for c in range(NCORES):
        qcols = np.asarray(wqkv[:, c * QF:(c + 1) * QF])
        kcols = np.asarray(wqkv[:, q_sz + c * HD:q_sz + (c + 1) * HD])
        vcols = np.asarray(
            wqkv[:, q_sz + NCORES * HD + c * HD:
                 q_sz + NCORES * HD + (c + 1) * HD])
        wqkv_c = np.concatenate([qcols, kcols, vcols], axis=1).astype(bf16)
        wo_c = np.ascontiguousarray(
            np.asarray(wo[:, c * ODPC:(c + 1) * ODPC])).astype(bf16)
        in_maps.append({
            "xt": xt, "wqkv": wqkv_c, "wo": wo_c,
            "sincos2": sincos2, "maskblk": maskblk, "eye": eye,
        })
    return in_maps, block_cls, n_mixed, qc_mask


_CACHE = {}


def run_distributed(x, wqkv, wo, sincos, full_causal_mask, start_pos,
                    NB, S, D, HPC, NCORES, trace=False, tmpdir=None):
    in_maps, block_cls, n_mixed, qc_mask = _host_prep(
        x, wqkv, wo, sincos, full_causal_mask, start_pos,
        NB, S, D, HPC, NCORES)
    key = (NB, S, D, HPC, NCORES,
           tuple(sorted((k, v) for k, v in block_cls.items())))
    if key not in _CACHE:
        _CACHE[key] = build_graph(NB, S, D, HPC, NCORES, block_cls, n_mixed,
                                  qc_mask)
    nc = _CACHE[key]
    res = run_bass_kernel_spmd(nc, in_maps, list(range(NCORES)), trace=trace,
                               tmpdir=tmpdir)
    TOK = NB * S
    out = np.empty((TOK, D), dtype=np.float32)
    ODPC = D // NCORES
    for c in range(NCORES):
        out[:, c * ODPC:(c + 1) * ODPC] = res.results[c]["out"].T
    return out.reshape(NB, S, D), res


def kernel(x, wqkv, wo, sincos, cache_k, cache_v, full_causal_mask,
           start_pos) -> np.ndarray:
    x = np.asarray(x)
    start_pos = int(np.asarray(start_pos))
    B, S_, D_ = x.shape
    assert start_pos == 0, "prefill-only kernel (seq fills the whole cache)"
    out, _ = run_distributed(
        x, np.asarray(wqkv), np.asarray(wo), np.asarray(sincos),
        np.asarray(full_causal_mask), start_pos,
        NB=B, S=S_, D=D_, HPC=4, NCORES=8)
    return out
